# revision 15
# baseline (speedup 1.0000x reference)
"""Trainium2 Bass kernel for nn_DA_conv (degradation-aware dynamic-filter conv).

kernel(**inputs) takes FULL inputs (as from setup_inputs()), shards batch
B=16 across 8 NeuronCores (2 batches/core), runs one SPMD Bass program on
cores 0-7, gathers the full [16,64,128,128] fp32 output.

v3 design (vs v2.2 baseline 493us):
  - PE fast-state aware: 512-col matmuls run at ~216ns only when stationary
    churn is low. All matmuls are full-width single passes (no quadrant
    pairs); the 9-tap conv accumulation chain uses one constant
    block-diagonal stationary; every ddf broadcast uses ONE constant 2-row
    selector stationary, with per-tap ksp rows realigned to partition base
    0 by small SBUF-SBUF DMAs (the BIR verifier requires 32-aligned
    LDWEIGHTS partition bases).
  - kc folded into the PSUM evacuation (ACT Identity scale AP / DVE
    tensor_scalar_mul), so all elementwise muls run bf16 SBUF at DVE 2x.
  - channel max replaced by log-sum-exp: mx ~ 6 + log(sum exp(12(x-6)))/12
    via ACT exp + ones-matmul + ACT ln; kills the serialized DMA fold
    chains. 1/12, +6, 1/64 scales folded into wsa/bsa host-side.
  - 8-row chunks, software pipeline lag 1, PE warmup burst at start.
"""

import sys

sys.path.insert(0, "/opt/trn_rl_repo")

import dataclasses

import numpy as np
import ml_dtypes

import concourse.bass as bass
import concourse.tile as tile
from concourse import bacc, mybir
from concourse.bass_utils import run_bass_kernel_spmd

F32 = mybir.dt.float32
BF16 = mybir.dt.bfloat16
AF = mybir.ActivationFunctionType
OP = mybir.AluOpType

B, C, H, W = 16, 64, 128, 128
KK = 9
HW = H * W
NCORES = 8
BPC = B // NCORES          # batches per core
RC = 8                     # image rows per chunk
NCH = H // RC              # 16 chunks
F = RC * W                 # 1024 pixels per chunk
Q = 512                    # psum-bank quantum
PW = 132                   # padded row width
PR = 130                   # padded rows
PADN = PR * PW
NT = 10 * PW               # flat tap window length (8 rows + 2 halo)
MSS = PADN + 4 * PW        # map tile cols (1 leading pad row + overread)
LSET = 12.0                # LSE sharpness
LSEB = 2.0                 # LSE offset

EVAC_DVE = {1, 3, 5, 7}    # taps whose PSUM evac runs on DVE (rest ACT)
GP_MUL = {2, 5, 6, 8}      # taps whose elementwise mul runs on GPSIMD
ACT_MUL = set()            # taps evac'd on ACT, mul on DVE


def _leaky(v):
    return np.where(v >= 0, v, 0.1 * v)


def _build_program():
    nc = bacc.Bacc("TRN2", target_bir_lowering=False, debug=False,
                   num_devices=NCORES)

    def din(name, shape, dt=F32):
        return nc.dram_tensor(name, shape, dt, kind="ExternalInput").ap()

    x0_d = din("x0b", [128, HW], BF16)
    x2_d = din("x2b", [128, HW], BF16)
    sel2_d = din("sel2", [2, 128], BF16)
    wks1bd_d = din("wks1bd", [128, 128], BF16)
    wks2bd_d = din("wks2bd", [128, 20], BF16)
    wbd_d = din("wbd", [128, 128], BF16)
    ones2_d = din("ones2", [128, 2], BF16)
    ones34_d = din("ones34", [128, 34], BF16)
    wsa36_d = din("wsa36", [36, 2], BF16)
    kc10_d = din("kc10", [128, KK])
    bks1_d = din("bks1", [128, 1])
    bks2i_d = din("bks2i", [20, 1])
    bsat_d = din("bsat", [2, 1])
    attb_d = din("attb", [128, 1])
    bconv_d = din("bconv", [128, 1])
    expb_d = din("expb", [128, 1])
    out_d = nc.dram_tensor("outb", [128, HW], BF16, kind="ExternalOutput").ap()

    with tile.TileContext(nc) as tc:
        with (
            tc.tile_pool(name="persist", bufs=1) as pp,
            tc.tile_pool(name="ring2", bufs=2) as r2,
            tc.tile_pool(name="ring3", bufs=3) as r3,
            tc.tile_pool(name="ringz", bufs=12) as rz,
            tc.tile_pool(name="ringt", bufs=12) as rt,
            tc.tile_pool(name="psK", bufs=3, space=bass.MemorySpace.PSUM) as psK,
            tc.tile_pool(name="psF", bufs=2, space=bass.MemorySpace.PSUM) as psF,
            tc.tile_pool(name="psX", bufs=3, space=bass.MemorySpace.PSUM) as psX,
        ):
            pad1 = pp.tile([128, PADN], BF16)
            pad2 = pp.tile([128, PADN], BF16)
            mssc = pp.tile([34, MSS], BF16)
            sel2 = pp.tile([2, 128], BF16)
            wks1bd = pp.tile([128, 128], BF16)
            wks2bd = pp.tile([128, 20], BF16)
            wbd = pp.tile([128, 128], BF16)
            ones2 = pp.tile([128, 2], BF16)
            ones34 = pp.tile([128, 34], BF16)
            wsa36 = pp.tile([36, 2], BF16)
            kc10 = pp.tile([128, KK], F32)
            bks1 = pp.tile([128, 1], F32)
            bks2i = pp.tile([20, 1], F32)
            bsat = pp.tile([2, 1], F32)
            attb = pp.tile([128, 1], F32)
            bconv = pp.tile([128, 1], F32)
            expb = pp.tile([128, 1], F32)
            wup = pp.tile([128, 640], BF16)

            MM = nc.tensor.matmul
            sdma = nc.sync.dma_start

            # PE warmup: flip the PE into its fast state before real work
            nc.gpsimd.memset(wup[:], 0.0)
            for r in range(8):
                wps = psX.tile([128, Q], F32, tag="psX", name="wup")
                MM(wps[:, :], wup[:, 0:128], wup[:, 128:640], start=True,
                   stop=True, tile_position=(0, 0))

            sdma(sel2[:], sel2_d[:])
            sdma(wks1bd[:], wks1bd_d[:])
            sdma(wks2bd[:], wks2bd_d[:])
            sdma(wbd[:], wbd_d[:])
            sdma(ones2[:], ones2_d[:])
            sdma(ones34[:], ones34_d[:])
            sdma(wsa36[:], wsa36_d[:])
            sdma(kc10[:], kc10_d[:])
            sdma(bks1[:], bks1_d[:])
            sdma(bks2i[:], bks2i_d[:])
            sdma(bsat[:], bsat_d[:])
            sdma(attb[:], attb_d[:])
            sdma(bconv[:], bconv_d[:])
            sdma(expb[:], expb_d[:])

            p1v = pad1.rearrange("p (r w) -> p r w", w=PW)
            p2v = pad2.rearrange("p (r w) -> p r w", w=PW)
            mscv = mssc.rearrange("p (r w) -> p r w", w=PW)

            # zero pad borders (interior fully DMA-overwritten) and map tile
            nc.gpsimd.memset(p1v[:, :, 0:2], 0.0)
            nc.gpsimd.memset(p1v[:, :, 130:132], 0.0)
            nc.gpsimd.memset(p1v[:, 0:1, :], 0.0)
            nc.gpsimd.memset(p1v[:, 129:130, :], 0.0)
            nc.gpsimd.memset(p2v[:, :, 0:3], 0.0)
            nc.gpsimd.memset(p2v[:, :, 131:132], 0.0)
            nc.gpsimd.memset(p2v[:, 0:1, :], 0.0)
            nc.gpsimd.memset(p2v[:, 129:130, :], 0.0)
            # zero only map border cells (interior rewritten every chunk);
            # mx-map border decodes to exactly mx=0 at conv borders
            nc.vector.memset(mscv[0:2, 0:2, :], 0.0)
            nc.vector.memset(mscv[0:2, 129:134, :], 0.0)
            nc.vector.memset(mscv[0:2, :, 0:2], 0.0)
            nc.vector.memset(mscv[0:2, :, 130:132], 0.0)
            nc.vector.memset(mscv[32:34, 0:2, :], -LSET * LSEB)
            nc.vector.memset(mscv[32:34, 129:134, :], -LSET * LSEB)
            nc.vector.memset(mscv[32:34, :, 0:2], -LSET * LSEB)
            nc.vector.memset(mscv[32:34, :, 130:132], -LSET * LSEB)

            kspEs = {}
            kspTs = {}
            x2ts = {}

            def strided_src(t, row, off, dims):
                a = t[row:row + 1, off:off + 1]
                return dataclasses.replace(a, ap=[[t.ap[0][0], 1]] + dims)

            for ch in range(NCH + 2):
                # ---- A: issue input DMAs for chunk ch ----
                if ch < NCH:
                    r0 = RC * ch
                    csl = slice(ch * F, (ch + 1) * F)
                    sdma(p1v[:, r0 + 1:r0 + 9, 2:130], x0_d[:, csl])
                    sdma(p2v[:, r0 + 1:r0 + 9, 3:131], x0_d[:, csl])
                    x2t = r2.tile([128, F], BF16, tag="x2t")
                    sdma(x2t[:], x2_d[:, csl])
                    x2ts[ch] = x2t

                # ---- B: maps + hidden + ksp for chunk cm = ch-1 ----
                if 1 <= ch <= NCH:
                    cm = ch - 1
                    r0 = RC * cm
                    x2t = x2ts.pop(cm)
                    kspE = r3.tile([20, F], BF16, tag="kspE")
                    kspEs[cm] = kspE
                    et = r2.tile([128, F], BF16, tag="et")
                    etv = et.rearrange("p (r w) -> p r w", w=W)
                    hsb = r2.tile([128, F], BF16, tag="hsb")
                    hpss = []
                    for q in range(2):
                        qs = slice(q * Q, (q + 1) * Q)
                        hps = psX.tile([128, Q], F32, tag="psX", name="hps")
                        MM(hps[:, :], wks1bd[:, :], x2t[:, qs], start=True,
                           stop=True, tile_position=(0, 0))
                        hpss.append(hps)
                    for q in range(2):
                        qs = slice(q * Q, (q + 1) * Q)
                        nc.scalar.activation(hsb[:, qs], hpss[q][:, :],
                                             AF.Prelu, bias=bks1[:, 0:1],
                                             alpha=0.1)
                    for q in range(2):
                        qs = slice(q * Q, (q + 1) * Q)
                        kps = psX.tile([128, Q], F32, tag="psX", name="kps")
                        MM(kps[0:20, :], wks2bd[:, :], hsb[:, qs], start=True,
                           stop=True, tile_position=(0, 0))
                        nc.scalar.activation(kspE[0:18, qs], kps[0:18, :],
                                             AF.Identity,
                                             bias=bks2i[0:18, 0:1])
                    nc.scalar.activation(etv[:, :, :],
                                         p1v[:, r0 + 1:r0 + 9, 2:130],
                                         AF.Exp, bias=expb[:, 0:1], scale=LSET)
                    for q in range(2):
                        rq = r0 + 4 * q
                        ss = psX.tile([128, Q], F32, tag="psX", name="ss")
                        MM(ss[0:2, :], ones2[:, :],
                           p1v[:, rq + 1:rq + 5, 2:130], start=True,
                           stop=True, tile_position=(0, 0))
                        nc.vector.tensor_copy(
                            mscv[0:2, rq + 2:rq + 6, 2:130],
                            ss[0:2, :].rearrange("p (r w) -> p r w", w=W))
                    for q in range(2):
                        rq = r0 + 4 * q
                        ss2 = psX.tile([128, Q], F32, tag="psX", name="ss2")
                        MM(ss2[0:34, :], ones34[:, :], et[:, q * Q:(q + 1) * Q],
                           start=True, stop=True, tile_position=(0, 0))
                        nc.scalar.activation(
                            mscv[32:34, rq + 2:rq + 6, 2:130],
                            ss2[32:34, :].rearrange("p (r w) -> p r w", w=W),
                            AF.Ln)

                # ---- C: taps + sa + ddf for chunk cd = ch-2 ----
                if ch >= 2:
                    cd = ch - 2
                    rp = RC * cd
                    tap36 = r2.tile([36, NT], BF16, tag="tap")
                    for m in range(4):
                        mr = m if m < 2 else m + 30
                        for j in range(3):
                            base = (m * 3 + j) * 3
                            src = strided_src(mssc, mr,
                                              (rp + 1) * PW + (j - 1),
                                              [[PW, 3], [1, NT]])
                            sdma(tap36[base:base + 3, :], src)
                    tapv = tap36.rearrange("p (r w) -> p r w", w=PW)
                    ktaps = kspTs.pop(cd)
                    outst = r2.tile([128, F], BF16, tag="outst")

                    def padview(t, q=None):
                        if t < KK:
                            i, j = divmod(t, 3)
                        else:
                            i, j = 1, 1
                        if j == 1:
                            srcv, joff = p1v, 2
                        else:
                            srcv, joff = p2v, j + 2
                        if q is None:
                            return srcv[:, rp + i:rp + i + 8, joff:joff + W]
                        return srcv[:, rp + i + 4 * q:rp + i + 4 * q + 4,
                                    joff:joff + W]

                    zs = {}

                    def bcast_tap(t):
                        z = rz.tile([128, F], BF16, tag="z")
                        zv = z.rearrange("p (r w) -> p r w", w=W)
                        gp = t in GP_MUL
                        sa = t == KK or t in ACT_MUL
                        kt = None
                        if gp or sa:
                            kt = r3.tile([128, F], BF16, tag="kt", name="kt")
                        for q in range(2):
                            qs = slice(q * Q, (q + 1) * Q)
                            ktp = psK.tile([128, Q], F32, tag="psK",
                                           name="ktp")
                            MM(ktp[:, :], sel2[0:2, :], ktaps[t][0:2, qs],
                               start=True, stop=True, tile_position=(0, 0))
                            if t == KK:
                                nc.scalar.activation(kt[:, qs], ktp[:, :],
                                                     AF.Identity,
                                                     bias=attb[:, 0:1])
                            elif gp or sa:
                                nc.scalar.activation(kt[:, qs], ktp[:, :],
                                                     AF.Identity,
                                                     scale=kc10[:, t:t + 1])
                            else:
                                nc.vector.scalar_tensor_tensor(
                                    zv[:, 4 * q:4 * q + 4, :], padview(t, q),
                                    kc10[:, t:t + 1],
                                    ktp[:, :].rearrange("p (r w) -> p r w",
                                                        w=W),
                                    OP.mult, OP.mult)
                        if gp:
                            nc.gpsimd.tensor_tensor(
                                zv, padview(t),
                                kt.rearrange("p (r w) -> p r w", w=W),
                                OP.mult)
                        elif sa:
                            nc.vector.tensor_mul(
                                zv, padview(t),
                                kt.rearrange("p (r w) -> p r w", w=W))
                        zs[t] = z

                    def conv_tap(t):
                        z = zs[t]
                        for q in range(2):
                            MM(fq[q][:, :], wbd[:, :], z[:, q * Q:(q + 1) * Q],
                               start=(t == 0), stop=(t == 8),
                               tile_position=(0, 0), skip_group_check=True)

                    for t in (0, 2, 1, 6, 3, 8, 4, 5, 7):  # S/G alternating
                        bcast_tap(t)
                    # sa conv + sigmoid via exp/ln/exp (stays in one
                    # ACT table set with Exp/Ln/Identity/Prelu)
                    sat = r2.tile([2, F], BF16, tag="sat")
                    sau = r2.tile([2, F], BF16, tag="sau")
                    saw = r2.tile([2, F], BF16, tag="saw")
                    for q in range(2):
                        saps = psF.tile([128, Q], F32, tag="psF", name="saps")
                        MM(saps[0:2, :], wsa36[:, :],
                           tapv[:, 4 * q:4 * q + 4, 2:130], start=True,
                           stop=True, tile_position=(0, 0))
                        nc.scalar.activation(sau[0:2, q * Q:(q + 1) * Q],
                                             saps[0:2, :], AF.Exp,
                                             bias=bsat[0:2, 0:1], scale=-1.0)
                    nc.scalar.activation(saw[:], sau[:], AF.Ln, bias=1.0)
                    nc.scalar.activation(sat[:], saw[:], AF.Exp, scale=-1.0)
                    ktap9 = rt.tile([2, F], BF16, tag="ktap")
                    sdma(ktap9[:], sat[:])
                    ktaps.append(ktap9)
                    fq = [psF.tile([128, Q], F32, tag="psF", name="fq")
                          for _ in range(2)]
                    for t in (0, 1, 3, 4, 7, 2, 5, 6):
                        conv_tap(t)
                    bcast_tap(KK)
                    conv_tap(8)
                    t2 = zs[KK]
                    for q in range(2):
                        osl = slice(q * Q, (q + 1) * Q)
                        nc.vector.scalar_tensor_tensor(
                            outst[:, osl], fq[q][:, :], bconv[:, 0:1],
                            t2[:, osl], OP.add, OP.add)
                    sdma(out_d[:, cd * F:(cd + 1) * F], outst[:])

                # ---- D: per-tap ksp realignment for cm (consumed next iter) ----
                if 1 <= ch <= NCH:
                    cm = ch - 1
                    kspE = kspEs.pop(cm)
                    ktaps = []
                    for t in range(KK):
                        ktap = rt.tile([2, F], BF16, tag="ktap")
                        sdma(ktap[:], kspE[2 * t:2 * t + 2, :])
                        ktaps.append(ktap)
                    kspTs[cm] = ktaps

    nc.compile()
    return nc


_CACHED = {}


def _get_program():
    if "nc" not in _CACHED:
        _CACHED["nc"] = _build_program()
    return _CACHED["nc"]


def make_in_maps(x0, x1, x2, Wkc1, Wkc2, Wks1, bks1, Wks2, bks2,
                 Wconv, bconv, Wca1, Wca2, Wsa, bsa):
    bf = ml_dtypes.bfloat16
    x0 = np.asarray(x0, np.float32)
    x1 = np.asarray(x1, np.float32)
    x2 = np.asarray(x2, np.float32)
    Wsa_np = np.asarray(Wsa, np.float32)[0]        # [2 src, 3 i, 3 j]

    kc = (_leaky(x1 @ np.asarray(Wkc1, np.float32))
          @ np.asarray(Wkc2, np.float32)).reshape(B, C, KK)
    att = 1.0 / (1.0 + np.exp(-(_leaky(x1 @ np.asarray(Wca1, np.float32))
                                @ np.asarray(Wca2, np.float32))))

    sel2_np = np.zeros((2, 128), np.float32)
    sel2_np[0, 0:64] = 1.0
    sel2_np[1, 64:128] = 1.0

    wks1_np = np.asarray(Wks1, np.float32)
    wks1bd_np = np.zeros((128, 128), np.float32)
    wks1bd_np[0:64, 0:64] = wks1_np
    wks1bd_np[64:128, 64:128] = wks1_np

    wks2_np = np.asarray(Wks2, np.float32)
    wks2bd_np = np.zeros((128, 20), np.float32)
    for t in range(KK):
        wks2bd_np[0:64, 2 * t] = wks2_np[:, t]
        wks2bd_np[64:128, 2 * t + 1] = wks2_np[:, t]

    wconv_np = np.asarray(Wconv, np.float32)
    wbd_np = np.zeros((128, 128), np.float32)
    wbd_np[0:64, 0:64] = wconv_np
    wbd_np[64:128, 64:128] = wconv_np

    ones2_np = np.zeros((128, 2), np.float32)
    ones2_np[0:64, 0] = 1.0
    ones2_np[64:128, 1] = 1.0
    ones34_np = np.zeros((128, 34), np.float32)
    ones34_np[0:64, 32] = 1.0
    ones34_np[64:128, 33] = 1.0

    # sa conv as 36-row matmul; rows (m, j, i): m 0/1 = sum-x (av) b0/b1,
    # m 2/3 = lse (mx) b0/b1; av rows carry 1/64, mx rows 1/LSET
    wsa36_np = np.zeros((36, 2), np.float32)
    for m in range(4):
        b = m % 2
        ch_sa = 1 if m < 2 else 0            # av is sa channel 1, mx is 0
        sc = (1.0 / 64.0) if m < 2 else (1.0 / LSET)
        for j in range(3):
            for i in range(3):
                wsa36_np[(m * 3 + j) * 3 + i, b] = Wsa_np[ch_sa, i, j] * sc

    bsa_f = float(np.asarray(bsa, np.float32)[0]) + \
        LSEB * float(Wsa_np[0].sum())        # fold the +6 LSE offset
    bsat_np = np.full((2, 1), -bsa_f, np.float32)

    bks1_np = np.tile(np.asarray(bks1, np.float32), 2).reshape(128, 1)
    bks2_np = np.asarray(bks2, np.float32)
    bks2i_np = np.zeros((20, 1), np.float32)
    for t in range(KK):
        bks2i_np[2 * t, 0] = bks2_np[t]
        bks2i_np[2 * t + 1, 0] = bks2_np[t]
    bconv_np = np.ascontiguousarray(
        np.tile(np.asarray(bconv, np.float32), 2).reshape(128, 1))
    expb_np = np.full((128, 1), -LSET * LSEB, np.float32)

    shared = {
        "sel2": sel2_np.astype(bf), "wks1bd": wks1bd_np.astype(bf),
        "wks2bd": wks2bd_np.astype(bf), "wbd": wbd_np.astype(bf),
        "ones2": ones2_np.astype(bf), "ones34": ones34_np.astype(bf),
        "wsa36": wsa36_np.astype(bf), "bks1": bks1_np, "bks2i": bks2i_np,
        "bsat": bsat_np, "bconv": bconv_np, "expb": expb_np,
    }

    in_maps = []
    for cid in range(NCORES):
        bsl = slice(BPC * cid, BPC * (cid + 1))
        m = dict(shared)
        m["x0b"] = np.ascontiguousarray(x0[bsl].reshape(128, HW).astype(bf))
        m["x2b"] = np.ascontiguousarray(x2[bsl].reshape(128, HW).astype(bf))
        kc10_np = np.empty((128, KK), np.float32)
        kc10_np[0:64, :] = kc[BPC * cid]
        kc10_np[64:128, :] = kc[BPC * cid + 1]
        m["kc10"] = kc10_np
        ab = np.empty((128, 1), np.float32)
        ab[0:64, 0] = att[BPC * cid] + 1.0
        ab[64:128, 0] = att[BPC * cid + 1] + 1.0
        m["attb"] = ab
        in_maps.append(m)
    return in_maps


def kernel(**inputs):
    in_maps = make_in_maps(**inputs)
    nc = _get_program()
    res = run_bass_kernel_spmd(nc, in_maps, list(range(NCORES)))
    out = np.empty((B, C, H, W), np.float32)
    for cid in range(NCORES):
        out[BPC * cid:BPC * (cid + 1)] = \
            res.results[cid]["outb"].astype(np.float32).reshape(BPC, C, H, W)
    return out


if __name__ == "__main__":
    _get_program()
    print("program built and compiled OK")


# revision 16
# speedup vs baseline: 1.0070x; 1.0070x over previous
"""Trainium2 Bass kernel for nn_DA_conv (degradation-aware dynamic-filter conv).

kernel(**inputs) takes FULL inputs (as from setup_inputs()), shards batch
B=16 across 8 NeuronCores (2 batches/core), runs one SPMD Bass program on
cores 0-7, gathers the full [16,64,128,128] fp32 output.

v3 design (vs v2.2 baseline 493us):
  - PE fast-state aware: 512-col matmuls run at ~216ns only when stationary
    churn is low. All matmuls are full-width single passes (no quadrant
    pairs); the 9-tap conv accumulation chain uses one constant
    block-diagonal stationary; every ddf broadcast uses ONE constant 2-row
    selector stationary, with per-tap ksp rows realigned to partition base
    0 by small SBUF-SBUF DMAs (the BIR verifier requires 32-aligned
    LDWEIGHTS partition bases).
  - kc folded into the PSUM evacuation (ACT Identity scale AP / DVE
    tensor_scalar_mul), so all elementwise muls run bf16 SBUF at DVE 2x.
  - channel max replaced by log-sum-exp: mx ~ 6 + log(sum exp(12(x-6)))/12
    via ACT exp + ones-matmul + ACT ln; kills the serialized DMA fold
    chains. 1/12, +6, 1/64 scales folded into wsa/bsa host-side.
  - 8-row chunks, software pipeline lag 1, PE warmup burst at start.
"""

import sys

sys.path.insert(0, "/opt/trn_rl_repo")

import dataclasses

import numpy as np
import ml_dtypes

import concourse.bass as bass
import concourse.tile as tile
from concourse import bacc, mybir
from concourse.bass_utils import run_bass_kernel_spmd

F32 = mybir.dt.float32
BF16 = mybir.dt.bfloat16
AF = mybir.ActivationFunctionType
OP = mybir.AluOpType

B, C, H, W = 16, 64, 128, 128
KK = 9
HW = H * W
NCORES = 8
BPC = B // NCORES          # batches per core
RC = 8                     # image rows per chunk
NCH = H // RC              # 16 chunks
F = RC * W                 # 1024 pixels per chunk
Q = 512                    # psum-bank quantum
PW = 132                   # padded row width
PR = 130                   # padded rows
PADN = PR * PW
NT = 10 * PW               # flat tap window length (8 rows + 2 halo)
MSS = PADN + 4 * PW        # map tile cols (1 leading pad row + overread)
LSET = 12.0                # LSE sharpness
LSEB = 2.0                 # LSE offset

EVAC_DVE = {1, 3, 5, 7}    # taps whose PSUM evac runs on DVE (rest ACT)
GP_MUL = {2, 6, 8}         # taps whose elementwise mul runs on GPSIMD
ACT_MUL = set()            # taps evac'd on ACT, mul on DVE


def _leaky(v):
    return np.where(v >= 0, v, 0.1 * v)


def _build_program():
    nc = bacc.Bacc("TRN2", target_bir_lowering=False, debug=False,
                   num_devices=NCORES)

    def din(name, shape, dt=F32):
        return nc.dram_tensor(name, shape, dt, kind="ExternalInput").ap()

    x0_d = din("x0b", [128, HW], BF16)
    x2_d = din("x2b", [128, HW], BF16)
    sel2_d = din("sel2", [2, 128], BF16)
    wks1bd_d = din("wks1bd", [128, 128], BF16)
    wks2bd_d = din("wks2bd", [128, 20], BF16)
    wbd_d = din("wbd", [128, 128], BF16)
    ones2_d = din("ones2", [128, 2], BF16)
    ones34_d = din("ones34", [128, 34], BF16)
    wsa36_d = din("wsa36", [36, 2], BF16)
    kc10_d = din("kc10", [128, KK])
    bks1_d = din("bks1", [128, 1])
    bks2i_d = din("bks2i", [20, 1])
    bsat_d = din("bsat", [2, 1])
    attb_d = din("attb", [128, 1])
    bconv_d = din("bconv", [128, 1])
    expb_d = din("expb", [128, 1])
    out_d = nc.dram_tensor("outb", [128, HW], BF16, kind="ExternalOutput").ap()

    with tile.TileContext(nc) as tc:
        with (
            tc.tile_pool(name="persist", bufs=1) as pp,
            tc.tile_pool(name="ring2", bufs=2) as r2,
            tc.tile_pool(name="ring3", bufs=3) as r3,
            tc.tile_pool(name="ringz", bufs=12) as rz,
            tc.tile_pool(name="ringt", bufs=12) as rt,
            tc.tile_pool(name="psK", bufs=3, space=bass.MemorySpace.PSUM) as psK,
            tc.tile_pool(name="psF", bufs=2, space=bass.MemorySpace.PSUM) as psF,
            tc.tile_pool(name="psX", bufs=3, space=bass.MemorySpace.PSUM) as psX,
        ):
            pad1 = pp.tile([128, PADN], BF16)
            pad2 = pp.tile([128, PADN], BF16)
            mssc = pp.tile([34, MSS], BF16)
            sel2 = pp.tile([2, 128], BF16)
            wks1bd = pp.tile([128, 128], BF16)
            wks2bd = pp.tile([128, 20], BF16)
            wbd = pp.tile([128, 128], BF16)
            ones2 = pp.tile([128, 2], BF16)
            ones34 = pp.tile([128, 34], BF16)
            wsa36 = pp.tile([36, 2], BF16)
            kc10 = pp.tile([128, KK], F32)
            bks1 = pp.tile([128, 1], F32)
            bks2i = pp.tile([20, 1], F32)
            bsat = pp.tile([2, 1], F32)
            attb = pp.tile([128, 1], F32)
            bconv = pp.tile([128, 1], F32)
            expb = pp.tile([128, 1], F32)
            wup = pp.tile([128, 640], BF16)

            MM = nc.tensor.matmul
            sdma = nc.sync.dma_start

            # PE warmup: flip the PE into its fast state before real work
            nc.gpsimd.memset(wup[:], 0.0)
            for r in range(8):
                wps = psX.tile([128, Q], F32, tag="psX", name="wup")
                MM(wps[:, :], wup[:, 0:128], wup[:, 128:640], start=True,
                   stop=True, tile_position=(0, 0))

            sdma(sel2[:], sel2_d[:])
            sdma(wks1bd[:], wks1bd_d[:])
            sdma(wks2bd[:], wks2bd_d[:])
            sdma(wbd[:], wbd_d[:])
            sdma(ones2[:], ones2_d[:])
            sdma(ones34[:], ones34_d[:])
            sdma(wsa36[:], wsa36_d[:])
            sdma(kc10[:], kc10_d[:])
            sdma(bks1[:], bks1_d[:])
            sdma(bks2i[:], bks2i_d[:])
            sdma(bsat[:], bsat_d[:])
            sdma(attb[:], attb_d[:])
            sdma(bconv[:], bconv_d[:])
            sdma(expb[:], expb_d[:])

            p1v = pad1.rearrange("p (r w) -> p r w", w=PW)
            p2v = pad2.rearrange("p (r w) -> p r w", w=PW)
            mscv = mssc.rearrange("p (r w) -> p r w", w=PW)

            # zero pad borders (interior fully DMA-overwritten) and map tile
            nc.gpsimd.memset(p1v[:, :, 0:2], 0.0)
            nc.gpsimd.memset(p1v[:, :, 130:132], 0.0)
            nc.gpsimd.memset(p1v[:, 0:1, :], 0.0)
            nc.gpsimd.memset(p1v[:, 129:130, :], 0.0)
            nc.gpsimd.memset(p2v[:, :, 0:3], 0.0)
            nc.gpsimd.memset(p2v[:, :, 131:132], 0.0)
            nc.gpsimd.memset(p2v[:, 0:1, :], 0.0)
            nc.gpsimd.memset(p2v[:, 129:130, :], 0.0)
            # zero only map border cells (interior rewritten every chunk);
            # mx-map border decodes to exactly mx=0 at conv borders
            nc.vector.memset(mscv[0:2, 0:2, :], 0.0)
            nc.vector.memset(mscv[0:2, 129:134, :], 0.0)
            nc.vector.memset(mscv[0:2, :, 0:2], 0.0)
            nc.vector.memset(mscv[0:2, :, 130:132], 0.0)
            nc.vector.memset(mscv[32:34, 0:2, :], -LSET * LSEB)
            nc.vector.memset(mscv[32:34, 129:134, :], -LSET * LSEB)
            nc.vector.memset(mscv[32:34, :, 0:2], -LSET * LSEB)
            nc.vector.memset(mscv[32:34, :, 130:132], -LSET * LSEB)

            kspEs = {}
            kspTs = {}
            x2ts = {}

            def strided_src(t, row, off, dims):
                a = t[row:row + 1, off:off + 1]
                return dataclasses.replace(a, ap=[[t.ap[0][0], 1]] + dims)

            for ch in range(NCH + 2):
                # ---- A: issue input DMAs for chunk ch ----
                if ch < NCH:
                    r0 = RC * ch
                    csl = slice(ch * F, (ch + 1) * F)
                    sdma(p1v[:, r0 + 1:r0 + 9, 2:130], x0_d[:, csl])
                    sdma(p2v[:, r0 + 1:r0 + 9, 3:131], x0_d[:, csl])
                    x2t = r2.tile([128, F], BF16, tag="x2t")
                    sdma(x2t[:], x2_d[:, csl])
                    x2ts[ch] = x2t

                # ---- B: maps + hidden + ksp for chunk cm = ch-1 ----
                if 1 <= ch <= NCH:
                    cm = ch - 1
                    r0 = RC * cm
                    x2t = x2ts.pop(cm)
                    kspE = r3.tile([20, F], BF16, tag="kspE")
                    kspEs[cm] = kspE
                    et = r2.tile([128, F], BF16, tag="et")
                    etv = et.rearrange("p (r w) -> p r w", w=W)
                    hsb = r2.tile([128, F], BF16, tag="hsb")
                    hpss = []
                    for q in range(2):
                        qs = slice(q * Q, (q + 1) * Q)
                        hps = psX.tile([128, Q], F32, tag="psX", name="hps")
                        MM(hps[:, :], wks1bd[:, :], x2t[:, qs], start=True,
                           stop=True, tile_position=(0, 0))
                        hpss.append(hps)
                    for q in range(2):
                        qs = slice(q * Q, (q + 1) * Q)
                        nc.scalar.activation(hsb[:, qs], hpss[q][:, :],
                                             AF.Prelu, bias=bks1[:, 0:1],
                                             alpha=0.1)
                    for q in range(2):
                        qs = slice(q * Q, (q + 1) * Q)
                        kps = psX.tile([128, Q], F32, tag="psX", name="kps")
                        MM(kps[0:20, :], wks2bd[:, :], hsb[:, qs], start=True,
                           stop=True, tile_position=(0, 0))
                        nc.scalar.activation(kspE[0:18, qs], kps[0:18, :],
                                             AF.Identity,
                                             bias=bks2i[0:18, 0:1])
                    nc.scalar.activation(etv[:, :, :],
                                         p1v[:, r0 + 1:r0 + 9, 2:130],
                                         AF.Exp, bias=expb[:, 0:1], scale=LSET)
                    for q in range(2):
                        rq = r0 + 4 * q
                        ss = psX.tile([128, Q], F32, tag="psX", name="ss")
                        MM(ss[0:2, :], ones2[:, :],
                           p1v[:, rq + 1:rq + 5, 2:130], start=True,
                           stop=True, tile_position=(0, 0))
                        nc.vector.tensor_copy(
                            mscv[0:2, rq + 2:rq + 6, 2:130],
                            ss[0:2, :].rearrange("p (r w) -> p r w", w=W))
                    for q in range(2):
                        rq = r0 + 4 * q
                        ss2 = psX.tile([128, Q], F32, tag="psX", name="ss2")
                        MM(ss2[0:34, :], ones34[:, :], et[:, q * Q:(q + 1) * Q],
                           start=True, stop=True, tile_position=(0, 0))
                        nc.scalar.activation(
                            mscv[32:34, rq + 2:rq + 6, 2:130],
                            ss2[32:34, :].rearrange("p (r w) -> p r w", w=W),
                            AF.Ln)

                # ---- C: taps + sa + ddf for chunk cd = ch-2 ----
                if ch >= 2:
                    cd = ch - 2
                    rp = RC * cd
                    tap36 = r2.tile([36, NT], BF16, tag="tap")
                    for m in range(4):
                        mr = m if m < 2 else m + 30
                        for j in range(3):
                            base = (m * 3 + j) * 3
                            src = strided_src(mssc, mr,
                                              (rp + 1) * PW + (j - 1),
                                              [[PW, 3], [1, NT]])
                            sdma(tap36[base:base + 3, :], src)
                    tapv = tap36.rearrange("p (r w) -> p r w", w=PW)
                    ktaps = kspTs.pop(cd)
                    outst = r2.tile([128, F], BF16, tag="outst")

                    def padview(t, q=None):
                        if t < KK:
                            i, j = divmod(t, 3)
                        else:
                            i, j = 1, 1
                        if j == 1:
                            srcv, joff = p1v, 2
                        else:
                            srcv, joff = p2v, j + 2
                        if q is None:
                            return srcv[:, rp + i:rp + i + 8, joff:joff + W]
                        return srcv[:, rp + i + 4 * q:rp + i + 4 * q + 4,
                                    joff:joff + W]

                    zs = {}

                    def bcast_tap(t):
                        z = rz.tile([128, F], BF16, tag="z")
                        zv = z.rearrange("p (r w) -> p r w", w=W)
                        gp = t in GP_MUL
                        sa = t == KK or t in ACT_MUL
                        kt = None
                        if gp or sa:
                            kt = r3.tile([128, F], BF16, tag="kt", name="kt")
                        for q in range(2):
                            qs = slice(q * Q, (q + 1) * Q)
                            ktp = psK.tile([128, Q], F32, tag="psK",
                                           name="ktp")
                            MM(ktp[:, :], sel2[0:2, :], ktaps[t][0:2, qs],
                               start=True, stop=True, tile_position=(0, 0))
                            if t == KK:
                                nc.scalar.activation(kt[:, qs], ktp[:, :],
                                                     AF.Identity,
                                                     bias=attb[:, 0:1])
                            elif gp or sa:
                                nc.scalar.activation(kt[:, qs], ktp[:, :],
                                                     AF.Identity,
                                                     scale=kc10[:, t:t + 1])
                            else:
                                nc.vector.scalar_tensor_tensor(
                                    zv[:, 4 * q:4 * q + 4, :], padview(t, q),
                                    kc10[:, t:t + 1],
                                    ktp[:, :].rearrange("p (r w) -> p r w",
                                                        w=W),
                                    OP.mult, OP.mult)
                        if gp:
                            nc.gpsimd.tensor_tensor(
                                zv, padview(t),
                                kt.rearrange("p (r w) -> p r w", w=W),
                                OP.mult)
                        elif sa:
                            nc.vector.tensor_mul(
                                zv, padview(t),
                                kt.rearrange("p (r w) -> p r w", w=W))
                        zs[t] = z

                    def conv_tap(t):
                        z = zs[t]
                        for q in range(2):
                            MM(fq[q][:, :], wbd[:, :], z[:, q * Q:(q + 1) * Q],
                               start=(t == 0), stop=(t == 8),
                               tile_position=(0, 0), skip_group_check=True)

                    for t in (0, 2, 1, 6, 3, 8, 4, 5, 7):  # S/G alternating
                        bcast_tap(t)
                    # sa conv + sigmoid via exp/ln/exp (stays in one
                    # ACT table set with Exp/Ln/Identity/Prelu)
                    sat = r2.tile([2, F], BF16, tag="sat")
                    sau = r2.tile([2, F], BF16, tag="sau")
                    saw = r2.tile([2, F], BF16, tag="saw")
                    for q in range(2):
                        saps = psF.tile([128, Q], F32, tag="psF", name="saps")
                        MM(saps[0:2, :], wsa36[:, :],
                           tapv[:, 4 * q:4 * q + 4, 2:130], start=True,
                           stop=True, tile_position=(0, 0))
                        nc.scalar.activation(sau[0:2, q * Q:(q + 1) * Q],
                                             saps[0:2, :], AF.Exp,
                                             bias=bsat[0:2, 0:1], scale=-1.0)
                    nc.scalar.activation(saw[:], sau[:], AF.Ln, bias=1.0)
                    nc.scalar.activation(sat[:], saw[:], AF.Exp, scale=-1.0)
                    ktap9 = rt.tile([2, F], BF16, tag="ktap")
                    sdma(ktap9[:], sat[:])
                    ktaps.append(ktap9)
                    fq = [psF.tile([128, Q], F32, tag="psF", name="fq")
                          for _ in range(2)]
                    for t in (0, 1, 3, 4, 5, 7, 2, 6):
                        conv_tap(t)
                    bcast_tap(KK)
                    conv_tap(8)
                    t2 = zs[KK]
                    for q in range(2):
                        osl = slice(q * Q, (q + 1) * Q)
                        nc.vector.scalar_tensor_tensor(
                            outst[:, osl], fq[q][:, :], bconv[:, 0:1],
                            t2[:, osl], OP.add, OP.add)
                    sdma(out_d[:, cd * F:(cd + 1) * F], outst[:])

                # ---- D: per-tap ksp realignment for cm (consumed next iter) ----
                if 1 <= ch <= NCH:
                    cm = ch - 1
                    kspE = kspEs.pop(cm)
                    ktaps = []
                    for t in range(KK):
                        ktap = rt.tile([2, F], BF16, tag="ktap")
                        sdma(ktap[:], kspE[2 * t:2 * t + 2, :])
                        ktaps.append(ktap)
                    kspTs[cm] = ktaps

    nc.compile()
    return nc


_CACHED = {}


def _get_program():
    if "nc" not in _CACHED:
        _CACHED["nc"] = _build_program()
    return _CACHED["nc"]


def make_in_maps(x0, x1, x2, Wkc1, Wkc2, Wks1, bks1, Wks2, bks2,
                 Wconv, bconv, Wca1, Wca2, Wsa, bsa):
    bf = ml_dtypes.bfloat16
    x0 = np.asarray(x0, np.float32)
    x1 = np.asarray(x1, np.float32)
    x2 = np.asarray(x2, np.float32)
    Wsa_np = np.asarray(Wsa, np.float32)[0]        # [2 src, 3 i, 3 j]

    kc = (_leaky(x1 @ np.asarray(Wkc1, np.float32))
          @ np.asarray(Wkc2, np.float32)).reshape(B, C, KK)
    att = 1.0 / (1.0 + np.exp(-(_leaky(x1 @ np.asarray(Wca1, np.float32))
                                @ np.asarray(Wca2, np.float32))))

    sel2_np = np.zeros((2, 128), np.float32)
    sel2_np[0, 0:64] = 1.0
    sel2_np[1, 64:128] = 1.0

    wks1_np = np.asarray(Wks1, np.float32)
    wks1bd_np = np.zeros((128, 128), np.float32)
    wks1bd_np[0:64, 0:64] = wks1_np
    wks1bd_np[64:128, 64:128] = wks1_np

    wks2_np = np.asarray(Wks2, np.float32)
    wks2bd_np = np.zeros((128, 20), np.float32)
    for t in range(KK):
        wks2bd_np[0:64, 2 * t] = wks2_np[:, t]
        wks2bd_np[64:128, 2 * t + 1] = wks2_np[:, t]

    wconv_np = np.asarray(Wconv, np.float32)
    wbd_np = np.zeros((128, 128), np.float32)
    wbd_np[0:64, 0:64] = wconv_np
    wbd_np[64:128, 64:128] = wconv_np

    ones2_np = np.zeros((128, 2), np.float32)
    ones2_np[0:64, 0] = 1.0
    ones2_np[64:128, 1] = 1.0
    ones34_np = np.zeros((128, 34), np.float32)
    ones34_np[0:64, 32] = 1.0
    ones34_np[64:128, 33] = 1.0

    # sa conv as 36-row matmul; rows (m, j, i): m 0/1 = sum-x (av) b0/b1,
    # m 2/3 = lse (mx) b0/b1; av rows carry 1/64, mx rows 1/LSET
    wsa36_np = np.zeros((36, 2), np.float32)
    for m in range(4):
        b = m % 2
        ch_sa = 1 if m < 2 else 0            # av is sa channel 1, mx is 0
        sc = (1.0 / 64.0) if m < 2 else (1.0 / LSET)
        for j in range(3):
            for i in range(3):
                wsa36_np[(m * 3 + j) * 3 + i, b] = Wsa_np[ch_sa, i, j] * sc

    bsa_f = float(np.asarray(bsa, np.float32)[0]) + \
        LSEB * float(Wsa_np[0].sum())        # fold the +6 LSE offset
    bsat_np = np.full((2, 1), -bsa_f, np.float32)

    bks1_np = np.tile(np.asarray(bks1, np.float32), 2).reshape(128, 1)
    bks2_np = np.asarray(bks2, np.float32)
    bks2i_np = np.zeros((20, 1), np.float32)
    for t in range(KK):
        bks2i_np[2 * t, 0] = bks2_np[t]
        bks2i_np[2 * t + 1, 0] = bks2_np[t]
    bconv_np = np.ascontiguousarray(
        np.tile(np.asarray(bconv, np.float32), 2).reshape(128, 1))
    expb_np = np.full((128, 1), -LSET * LSEB, np.float32)

    shared = {
        "sel2": sel2_np.astype(bf), "wks1bd": wks1bd_np.astype(bf),
        "wks2bd": wks2bd_np.astype(bf), "wbd": wbd_np.astype(bf),
        "ones2": ones2_np.astype(bf), "ones34": ones34_np.astype(bf),
        "wsa36": wsa36_np.astype(bf), "bks1": bks1_np, "bks2i": bks2i_np,
        "bsat": bsat_np, "bconv": bconv_np, "expb": expb_np,
    }

    in_maps = []
    for cid in range(NCORES):
        bsl = slice(BPC * cid, BPC * (cid + 1))
        m = dict(shared)
        m["x0b"] = np.ascontiguousarray(x0[bsl].reshape(128, HW).astype(bf))
        m["x2b"] = np.ascontiguousarray(x2[bsl].reshape(128, HW).astype(bf))
        kc10_np = np.empty((128, KK), np.float32)
        kc10_np[0:64, :] = kc[BPC * cid]
        kc10_np[64:128, :] = kc[BPC * cid + 1]
        m["kc10"] = kc10_np
        ab = np.empty((128, 1), np.float32)
        ab[0:64, 0] = att[BPC * cid] + 1.0
        ab[64:128, 0] = att[BPC * cid + 1] + 1.0
        m["attb"] = ab
        in_maps.append(m)
    return in_maps


def kernel(**inputs):
    in_maps = make_in_maps(**inputs)
    nc = _get_program()
    res = run_bass_kernel_spmd(nc, in_maps, list(range(NCORES)))
    out = np.empty((B, C, H, W), np.float32)
    for cid in range(NCORES):
        out[BPC * cid:BPC * (cid + 1)] = \
            res.results[cid]["outb"].astype(np.float32).reshape(BPC, C, H, W)
    return out


if __name__ == "__main__":
    _get_program()
    print("program built and compiled OK")
